# revision 18
# baseline (speedup 1.0000x reference)
"""Trainium2 Bass kernel for nn_ECA (attention block + residual + LayerNorm).

Reference computation (per batch b):
    qkv = x @ qkv_w.T ; q,k,v per head
    attn = softmax((q @ k.T) * sqrt(D))
    x1 = attn @ v  -> concat heads -> @ proj_w.T + proj_b
    out = LayerNorm(x + x1) * gamma + beta     # eps 1e-5

Sharding: 8 cores = 4 batches x 2 query-halves. Each core receives the full
batch's tokens ("xk", rolled so its own 1024 query tokens are rows 0:1024),
computes K/V for all 2048 keys (duplicated across the 2 cores of a batch),
attention + proj + LN for its 1024 queries. No collectives.

Precision: logits need fp32-class accuracy (softmax here is near-argmax:
logit std ~64, top-2 gap ~13 -- tf32/bf16/fp16 single-pass rounding flips
argmaxes). Native fp32 matmul runs at 1/4 rate with poor pipelining, so Q/K/S
use fp16 limb-split matmuls instead: a = ah + al (fp16 high/low limbs),
a.b = ah.bh + ah.bl + al.bh (3 full-rate fp16 passes, ~2e-5 logit error).
V / P / attn@V / proj run in plain fp16 (smooth ~0.1% errors).
The sqrt(D)=8 scale is folded into wq on the host (exact, power of 2).
"""

import sys
from dataclasses import dataclass

import numpy as np

try:
    import concourse.bass  # noqa: F401
except ImportError:  # fresh dir without sitecustomize path
    sys.path.insert(0, "/opt/trn_rl_repo")


@dataclass(frozen=True)
class Cfg:
    Nk: int = 2048   # keys per core (full batch)
    Nq: int = 1024   # queries per core
    C: int = 768     # model dim (also total head dim H*D)
    H: int = 12
    D: int = 64

    @property
    def CH(self):
        return self.C // 128

    @property
    def G(self):
        return (self.H * self.D) // 128

    @property
    def TQ(self):
        return self.Nq // 128

    @property
    def TK(self):
        return self.Nk // 128

    @property
    def slabs(self):
        return self.Nk // self.Nq


def build_program(cfg: Cfg):
    import concourse.bass as bass
    import concourse.mybir as mybir
    import concourse.tile as tile
    from concourse import bacc
    from concourse.masks import make_identity

    f32 = mybir.dt.float32
    f16 = mybir.dt.float16
    ts = bass.ts
    Nk, Nq, C, H, D = cfg.Nk, cfg.Nq, cfg.C, cfg.H, cfg.D
    CH, G, TQ, TK = cfg.CH, cfg.G, cfg.TQ, cfg.TK
    QC = H * D
    assert QC % 128 == 0 and C % 128 == 0 and Nq % 128 == 0

    nc = bacc.Bacc("TRN2", target_bir_lowering=False, debug=False, num_devices=8)

    xk_d = nc.dram_tensor("xk", [Nk, C], f32, kind="ExternalInput")
    wqh_d = nc.dram_tensor("wq_hi", [C, QC], f16, kind="ExternalInput")
    wql_d = nc.dram_tensor("wq_lo", [C, QC], f16, kind="ExternalInput")
    wkh_d = nc.dram_tensor("wk_hi", [C, QC], f16, kind="ExternalInput")
    wkl_d = nc.dram_tensor("wk_lo", [C, QC], f16, kind="ExternalInput")
    wv_d = nc.dram_tensor("wv_h", [C, QC], f16, kind="ExternalInput")
    wp_d = nc.dram_tensor("wp_h", [QC, C], f16, kind="ExternalInput")
    vec_d = nc.dram_tensor("vecs", [3, C], f32, kind="ExternalInput")
    out_d = nc.dram_tensor("out", [Nq, C], f32, kind="ExternalOutput")

    SH = min(Nk, 1024)     # S psum half size
    NSH = Nk // SH
    J = min(512, Nq)       # psum free chunk
    JB = min(512, Nq)      # AV free chunk

    with tile.TileContext(nc) as tc:
        with tc.tile_pool(name="persist", bufs=1) as persist:
            idt_f = persist.tile([128, 128], f32, name="idt_f", tag="idt_f")
            make_identity(nc, idt_f[:])

            kh_t = [persist.tile([128, Nk], f16, name=f"kh{g}", tag=f"kh{g}") for g in range(G)]
            kl_t = [persist.tile([128, Nk], f16, name=f"kl{g}", tag=f"kl{g}") for g in range(G)]
            qh_t = [persist.tile([128, Nq], f16, name=f"qh{g}", tag=f"qh{g}") for g in range(G)]
            ql_t = [persist.tile([128, Nq], f16, name=f"ql{g}", tag=f"ql{g}") for g in range(G)]
            vb = [persist.tile([128, QC], f16, name=f"vb{t}", tag=f"vb{t}") for t in range(TK)]
            x1t = [persist.tile([128, Nq], f16, name=f"x1t{g}", tag=f"x1t{g}") for g in range(G)]

            # ---------------- Phase A: x^T limbs, Q^T, K^T, V ----------------
            with tc.tile_pool(name="pa_sb", bufs=2) as pa_sb, \
                 tc.tile_pool(name="pa_w", bufs=2) as pa_w, \
                 tc.tile_pool(name="pa_xt", bufs=1) as pa_xt, \
                 tc.tile_pool(name="pa_ps", bufs=2, space="PSUM") as pa_ps, \
                 tc.tile_pool(name="pa_psv", bufs=2, space="PSUM") as pa_psv, \
                 tc.tile_pool(name="pa_pst", bufs=2, space="PSUM") as pa_pst:

                for slab in range(cfg.slabs):
                    # transpose slab of x; evict as fp16 hi/lo limbs
                    xh = [pa_xt.tile([128, Nq], f16, name=f"xh{c}", tag=f"xh{c}") for c in range(CH)]
                    xl = [pa_xt.tile([128, Nq], f16, name=f"xl{c}", tag=f"xl{c}") for c in range(CH)]
                    for t in range(TQ):
                        xn = pa_sb.tile([128, C], f32, name="xn", tag="xn")
                        nc.sync.dma_start(xn[:], xk_d.ap()[(slab * TQ + t) * 128:(slab * TQ + t + 1) * 128, :])
                        for c in range(CH):
                            pst = pa_pst.tile([128, 128], f32, name="pst", tag="pst")
                            nc.tensor.transpose(pst[:], xn[:, ts(c, 128)], idt_f[:])
                            nc.scalar.copy(xh[c][:, ts(t, 128)], pst[:])
                            nc.vector.tensor_sub(xl[c][:, ts(t, 128)], pst[:], xh[c][:, ts(t, 128)])

                    # K^T limbs (and Q^T limbs on slab 0)
                    for g in range(G):
                        for (w_hi, w_lo, oh, ol, nj, off) in (
                            [(wkh_d, wkl_d, kh_t, kl_t, Nq // J, slab * Nq)] +
                            ([(wqh_d, wql_d, qh_t, ql_t, Nq // J, 0)] if slab == 0 else [])):
                            wgh = pa_w.tile([128, CH, 128], f16, name="wgh", tag="wgh")
                            wgl = pa_w.tile([128, CH, 128], f16, name="wgl", tag="wgl")
                            nc.sync.dma_start(wgh[:], w_hi.ap()[:, ts(g, 128)].rearrange("(c p) n -> p c n", p=128))
                            nc.sync.dma_start(wgl[:], w_lo.ap()[:, ts(g, 128)].rearrange("(c p) n -> p c n", p=128))
                            for j in range(nj):
                                ps = pa_ps.tile([128, J], f32, name="ps_qk", tag="ps_qk")
                                for c in range(CH):
                                    nc.tensor.matmul(ps[:], wgh[:, c, :], xh[c][:, ts(j, J)],
                                                     start=(c == 0), stop=False)
                                    nc.tensor.matmul(ps[:], wgh[:, c, :], xl[c][:, ts(j, J)],
                                                     start=False, stop=False)
                                    nc.tensor.matmul(ps[:], wgl[:, c, :], xh[c][:, ts(j, J)],
                                                     start=False, stop=(c == CH - 1))
                                sl = slice(off + j * J, off + (j + 1) * J)
                                nc.scalar.copy(oh[g][:, sl], ps[:])
                                nc.vector.tensor_sub(ol[g][:, sl], ps[:], oh[g][:, sl])

                    # V (token-major, fp16)
                    for vc_base in range(0, QC, 384):
                        vw = min(384, QC - vc_base)
                        wvg = pa_w.tile([128, CH, 384], f16, name="wvg", tag="wvg")
                        nc.sync.dma_start(
                            wvg[:, :, :vw],
                            wv_d.ap()[:, vc_base:vc_base + vw].rearrange("(c p) n -> p c n", p=128))
                        for t in range(TQ):
                            psv = pa_psv.tile([128, 384], f32, name="psv", tag="psv")
                            for c in range(CH):
                                nc.tensor.matmul(psv[:, :vw], xh[c][:, ts(t, 128)], wvg[:, c, :vw],
                                                 start=(c == 0), stop=(c == CH - 1))
                            nc.vector.tensor_copy(vb[slab * TQ + t][:, vc_base:vc_base + vw], psv[:, :vw])

            # ---------------- Phase B: attention ----------------
            # Blocks of BLK q-tiles; each block's AV is emitted one block
            # late so the PE never stalls on the DMA transposes (HAM stays
            # warm at 2.4GHz).
            BLK = min(4, TQ)
            with tc.tile_pool(name="pb_p", bufs=2) as pb_p, \
                 tc.tile_pool(name="pb_pn", bufs=2) as pb_pn, \
                 tc.tile_pool(name="pb_pth", bufs=3) as pb_pth, \
                 tc.tile_pool(name="pb_st", bufs=3) as pb_st, \
                 tc.tile_pool(name="pb_s", bufs=3, space="PSUM") as pb_s, \
                 tc.tile_pool(name="pb_x1", bufs=2, space="PSUM") as pb_x1:

                def emit_av(g, r, h, qb, pThb):
                    ps_x1 = pb_x1.tile([D, BLK * 128], f32, name="ps_x1", tag="ps_x1")
                    for k in range(TK):
                        nc.tensor.matmul(ps_x1[:],
                                         vb[k][:, h * D:(h + 1) * D],
                                         pThb[:, k, :, :].rearrange("p t q -> p (t q)"),
                                         start=(k == 0), stop=(k == TK - 1))
                    nc.vector.tensor_copy(
                        x1t[g][r:r + D, qb * BLK * 128:(qb + 1) * BLK * 128], ps_x1[:])

                pending = None
                for h in range(H):
                    g, r = divmod(h * D, 128)
                    for qb in range(TQ // BLK):
                        pThb = pb_pth.tile([128, TK, BLK, 128], f16, name="pThb", tag="pThb")
                        for tt in range(BLK):
                            t = qb * BLK + tt
                            qh_s = qh_t[g][r:r + D, ts(t, 128)]
                            ql_s = ql_t[g][r:r + D, ts(t, 128)]
                            s_tiles = []
                            nms = []
                            for sh in range(NSH):
                                ps_s = pb_s.tile([128, SH], f32, name="ps_s", tag="ps_s")
                                for j in range(SH // J):
                                    sl = slice(sh * SH + j * J, sh * SH + (j + 1) * J)
                                    nc.tensor.matmul(ps_s[:, ts(j, J)], qh_s, kh_t[g][r:r + D, sl],
                                                     start=True, stop=False)
                                    nc.tensor.matmul(ps_s[:, ts(j, J)], qh_s, kl_t[g][r:r + D, sl],
                                                     start=False, stop=False)
                                    nc.tensor.matmul(ps_s[:, ts(j, J)], ql_s, kh_t[g][r:r + D, sl],
                                                     start=False, stop=True)
                                nm = pb_st.tile([128, 1], f32, name="nm", tag="nm")
                                nc.vector.reduce_max(out=nm[:], in_=ps_s[:],
                                                     axis=mybir.AxisListType.X, negate=True)
                                s_tiles.append(ps_s)
                                nms.append(nm)
                            negm = pb_st.tile([128, 1], f32, name="negm", tag="negm")
                            if NSH > 1:
                                nc.vector.tensor_tensor(negm[:], nms[0][:], nms[1][:],
                                                        op=mybir.AluOpType.min)
                                for sh in range(2, NSH):
                                    nc.vector.tensor_tensor(negm[:], negm[:], nms[sh][:],
                                                            op=mybir.AluOpType.min)
                            else:
                                nc.vector.tensor_copy(negm[:], nms[0][:])

                            p_t = pb_p.tile([128, Nk], f32, name="p_t", tag="p_t")
                            l_parts = []
                            for sh in range(NSH):
                                l_i = pb_st.tile([128, 1], f32, name=f"l{sh}", tag=f"l{sh}")
                                nc.scalar.activation(p_t[:, sh * SH:(sh + 1) * SH], s_tiles[sh][:],
                                                     mybir.ActivationFunctionType.Exp,
                                                     bias=negm[:], accum_out=l_i[:])
                                l_parts.append(l_i)
                            l_tot = pb_st.tile([128, 1], f32, name="l_tot", tag="l_tot")
                            if NSH > 1:
                                nc.gpsimd.tensor_add(l_tot[:], l_parts[0][:], l_parts[1][:])
                                for sh in range(2, NSH):
                                    nc.gpsimd.tensor_add(l_tot[:], l_tot[:], l_parts[sh][:])
                            else:
                                nc.gpsimd.tensor_copy(l_tot[:], l_parts[0][:])
                            rl = pb_st.tile([128, 1], f32, name="rl", tag="rl")
                            nc.vector.reciprocal(rl[:], l_tot[:])
                            p_n = pb_pn.tile([128, Nk], f16, name="p_n", tag="p_n")
                            nc.gpsimd.tensor_scalar_mul(p_n[:], p_t[:], rl[:])
                            # blockwise transpose: pThb[p, k, tt, q] = p_n[q, k*128+p]
                            nc.sync.dma_start(pThb[:, :, tt, :], p_n[:], transpose=True)

                        if pending is not None:
                            emit_av(*pending)
                        pending = (g, r, h, qb, pThb)
                if pending is not None:
                    emit_av(*pending)

            # ---------------- Phase C: proj + residual + LayerNorm ----------------
            with tc.tile_pool(name="pc_w", bufs=1) as pc_w, \
                 tc.tile_pool(name="pc_sb", bufs=2) as pc_sb, \
                 tc.tile_pool(name="pc_st", bufs=3) as pc_st, \
                 tc.tile_pool(name="pc_ps", bufs=4, space="PSUM") as pc_ps, \
                 tc.tile_pool(name="pc_bc", bufs=3, space="PSUM") as pc_bc:

                # broadcast proj_b / gamma / beta across partitions via ones-matmul
                ones = pc_w.tile([1, 128], f32, name="ones", tag="ones")
                nc.gpsimd.memset(ones[:], 1.0)
                bc = []
                for vi in range(3):
                    vrow = pc_w.tile([1, C], f32, name=f"vrow{vi}", tag=f"vrow{vi}")
                    nc.sync.dma_start(vrow[:], vec_d.ap()[vi:vi + 1, :])
                    bct = pc_w.tile([128, C], f32, name=f"bc{vi}", tag=f"bc{vi}")
                    for j in range(0, C, 512):
                        w = min(512, C - j)
                        psb = pc_bc.tile([128, 512], f32, name="psb", tag="psb")
                        nc.tensor.matmul(psb[:, :w], ones[:], vrow[:, j:j + w],
                                         start=True, stop=True)
                        nc.scalar.copy(bct[:, j:j + w], psb[:, :w])
                    bc.append(bct)
                bias_bc, gam_bc, bet_bc = bc

                wpb = []
                for c in range(G):
                    wpc = pc_w.tile([128, C], f16, name=f"wpb{c}", tag=f"wpb{c}")
                    nc.sync.dma_start(wpc[:], wp_d.ap()[ts(c, 128), :])
                    wpb.append(wpc)

                eps_t = pc_w.tile([128, 1], f32, name="eps_t", tag="eps_t")
                nc.gpsimd.memset(eps_t[:], 1e-5)

                NSTAT = 256
                nsub = C // NSTAT
                for t in range(TQ):
                    pps = []
                    for j in range(0, C, 384):
                        w = min(384, C - j)
                        pp = pc_ps.tile([128, 384], f32, name="pp", tag="pp")
                        for c in range(G):
                            nc.tensor.matmul(pp[:, :w], x1t[c][:, ts(t, 128)], wpb[c][:, j:j + w],
                                             start=(c == 0), stop=(c == G - 1))
                        pps.append((j, w, pp))
                    xr = pc_sb.tile([128, C], f32, name="xr", tag="xr")
                    nc.sync.dma_start(xr[:], xk_d.ap()[ts(t, 128), :])
                    u = pc_sb.tile([128, C], f32, name="u", tag="u")
                    for (j, w, pp) in pps:
                        nc.vector.tensor_add(u[:, j:j + w], pp[:, :w], bias_bc[:, j:j + w])
                    nc.vector.tensor_add(u[:], u[:], xr[:])

                    stats = pc_st.tile([128, nsub, 6], f32, name="stats", tag="stats")
                    for s in range(nsub):
                        nc.vector.bn_stats(out=stats[:, s, :], in_=u[:, ts(s, NSTAT)])
                    mv = pc_st.tile([128, 2], f32, name="mv", tag="mv")
                    nc.vector.bn_aggr(out=mv[:], in_=stats[:])
                    rstd = pc_st.tile([128, 1], f32, name="rstd", tag="rstd")
                    nc.scalar.activation(rstd[:], mv[:, 1:2],
                                         mybir.ActivationFunctionType.Sqrt, bias=eps_t[:])
                    nc.vector.reciprocal(rstd[:], rstd[:])

                    of = pc_sb.tile([128, C], f32, name="of", tag="of")
                    nc.vector.tensor_scalar(out=of[:], in0=u[:], scalar1=mv[:, 0:1],
                                            scalar2=rstd[:], op0=mybir.AluOpType.subtract,
                                            op1=mybir.AluOpType.mult)
                    nc.vector.tensor_mul(of[:], of[:], gam_bc[:])
                    nc.vector.tensor_add(of[:], of[:], bet_bc[:])
                    nc.sync.dma_start(out_d.ap()[ts(t, 128), :], of[:])

    nc.compile()
    return nc


_CACHE = {}


def _get_program(cfg: Cfg):
    if cfg not in _CACHE:
        _CACHE[cfg] = build_program(cfg)
    return _CACHE[cfg]


def _split16(w):
    hi = w.astype(np.float16)
    lo = (w - hi.astype(np.float32)).astype(np.float16)
    return np.ascontiguousarray(hi), np.ascontiguousarray(lo)


def make_in_maps(x, qkv_w, proj_w, proj_b, ln_gamma, ln_beta, cfg: Cfg):
    """Host-side shard prep. Returns list of 8 in_maps."""
    C = cfg.C
    B = x.shape[0]
    wq_t = np.ascontiguousarray((qkv_w[0:C] * np.float32(cfg.D ** 0.5)).T)
    wk_t = np.ascontiguousarray(qkv_w[C:2 * C].T)
    wv_t = np.ascontiguousarray(qkv_w[2 * C:3 * C].T)
    wp_t = np.ascontiguousarray(proj_w.T)
    wq_hi, wq_lo = _split16(wq_t)
    wk_hi, wk_lo = _split16(wk_t)
    wv_h = wv_t.astype(np.float16)
    wp_h = wp_t.astype(np.float16)
    vecs = np.ascontiguousarray(np.stack([proj_b, ln_gamma, ln_beta]).astype(np.float32))
    in_maps = []
    for core in range(8):
        b, half = core // 2, core % 2
        b = min(b, B - 1)
        xb = np.asarray(x[b], dtype=np.float32)
        if half == 0:
            xkc = np.ascontiguousarray(xb)
        else:
            xkc = np.ascontiguousarray(np.concatenate([xb[cfg.Nq:], xb[:cfg.Nq]], axis=0))
        in_maps.append({"xk": xkc, "wq_hi": wq_hi, "wq_lo": wq_lo,
                        "wk_hi": wk_hi, "wk_lo": wk_lo, "wv_h": wv_h,
                        "wp_h": wp_h, "vecs": vecs})
    return in_maps


def kernel(x, qkv_w, proj_w, proj_b, ln_gamma, ln_beta):
    from concourse.bass_utils import run_bass_kernel_spmd

    cfg = Cfg()
    nc = _get_program(cfg)
    x = np.asarray(x, dtype=np.float32)
    in_maps = make_in_maps(x, np.asarray(qkv_w, np.float32), np.asarray(proj_w, np.float32),
                           np.asarray(proj_b, np.float32), np.asarray(ln_gamma, np.float32),
                           np.asarray(ln_beta, np.float32), cfg)
    res = run_bass_kernel_spmd(nc, in_maps, core_ids=list(range(8)))
    B, N, C = x.shape
    out = np.empty((B, N, C), dtype=np.float32)
    for core in range(8):
        b, half = core // 2, core % 2
        out[b, half * cfg.Nq:(half + 1) * cfg.Nq] = res.results[core]["out"]
    return out


# revision 19
# speedup vs baseline: 3.3814x; 3.3814x over previous
"""Trainium2 Bass kernel for nn_ECA (attention block + residual + LayerNorm).

Reference computation (per batch b):
    qkv = x @ qkv_w.T ; q,k,v per head
    attn = softmax((q @ k.T) * sqrt(D))
    x1 = attn @ v  -> concat heads -> @ proj_w.T + proj_b
    out = LayerNorm(x + x1) * gamma + beta     # eps 1e-5

Sharding: 8 cores = 4 batches x 2 query-halves. Each core receives the full
batch's tokens ("xk", rolled so its own 1024 query tokens are rows 0:1024),
computes K/V for all 2048 keys (duplicated across the 2 cores of a batch),
attention + proj + LN for its 1024 queries. No collectives.

Precision: logits need fp32-class accuracy (softmax here is near-argmax:
logit std ~64, top-2 gap ~13 -- tf32/bf16/fp16 single-pass rounding flips
argmaxes). Native fp32 matmul runs at 1/4 rate with poor pipelining, so Q/K/S
use fp16 limb-split matmuls instead: a = ah + al (fp16 high/low limbs),
a.b = ah.bh + ah.bl + al.bh (3 full-rate fp16 passes, ~2e-5 logit error).
V / P / attn@V / proj run in plain fp16 (smooth ~0.1% errors).
The sqrt(D)=8 scale is folded into wq on the host (exact, power of 2).
"""

import sys
from dataclasses import dataclass

import numpy as np

try:
    import concourse.bass  # noqa: F401
except ImportError:  # fresh dir without sitecustomize path
    sys.path.insert(0, "/opt/trn_rl_repo")


@dataclass(frozen=True)
class Cfg:
    Nk: int = 2048   # keys per core (full batch)
    Nq: int = 1024   # queries per core
    C: int = 768     # model dim (also total head dim H*D)
    H: int = 12
    D: int = 64

    @property
    def CH(self):
        return self.C // 128

    @property
    def G(self):
        return (self.H * self.D) // 128

    @property
    def TQ(self):
        return self.Nq // 128

    @property
    def TK(self):
        return self.Nk // 128

    @property
    def slabs(self):
        return self.Nk // self.Nq


def build_program(cfg: Cfg):
    import concourse.bass as bass
    import concourse.mybir as mybir
    import concourse.tile as tile
    from concourse import bacc
    from concourse.masks import make_identity

    f32 = mybir.dt.float32
    f16 = mybir.dt.float16
    ts = bass.ts
    Nk, Nq, C, H, D = cfg.Nk, cfg.Nq, cfg.C, cfg.H, cfg.D
    CH, G, TQ, TK = cfg.CH, cfg.G, cfg.TQ, cfg.TK
    QC = H * D
    assert QC % 128 == 0 and C % 128 == 0 and Nq % 128 == 0

    nc = bacc.Bacc("TRN2", target_bir_lowering=False, debug=False, num_devices=8)

    xk_d = nc.dram_tensor("xk", [Nk, C], f32, kind="ExternalInput")
    wqh_d = nc.dram_tensor("wq_hi", [C, QC], f16, kind="ExternalInput")
    wql_d = nc.dram_tensor("wq_lo", [C, QC], f16, kind="ExternalInput")
    wkh_d = nc.dram_tensor("wk_hi", [C, QC], f16, kind="ExternalInput")
    wkl_d = nc.dram_tensor("wk_lo", [C, QC], f16, kind="ExternalInput")
    wv_d = nc.dram_tensor("wv_h", [C, QC], f16, kind="ExternalInput")
    wp_d = nc.dram_tensor("wp_h", [QC, C], f16, kind="ExternalInput")
    vec_d = nc.dram_tensor("vecs", [3, C], f32, kind="ExternalInput")
    out_d = nc.dram_tensor("out", [Nq, C], f32, kind="ExternalOutput")

    SH = min(Nk, 1024)     # S psum half size
    NSH = Nk // SH
    J = min(512, Nq)       # psum free chunk
    JB = min(512, Nq)      # AV free chunk

    with tile.TileContext(nc) as tc:
        with tc.tile_pool(name="persist", bufs=1) as persist:
            idt_f = persist.tile([128, 128], f32, name="idt_f", tag="idt_f")
            make_identity(nc, idt_f[:])

            kh_t = [persist.tile([128, Nk], f16, name=f"kh{g}", tag=f"kh{g}") for g in range(G)]
            kl_t = [persist.tile([128, Nk], f16, name=f"kl{g}", tag=f"kl{g}") for g in range(G)]
            qh_t = [persist.tile([128, Nq], f16, name=f"qh{g}", tag=f"qh{g}") for g in range(G)]
            ql_t = [persist.tile([128, Nq], f16, name=f"ql{g}", tag=f"ql{g}") for g in range(G)]
            vb = [persist.tile([128, QC], f16, name=f"vb{t}", tag=f"vb{t}") for t in range(TK)]
            x1t = [persist.tile([128, Nq], f16, name=f"x1t{g}", tag=f"x1t{g}") for g in range(G)]

            # ---------------- Phase A: x^T limbs, Q^T, K^T, V ----------------
            with tc.tile_pool(name="pa_sb", bufs=2) as pa_sb, \
                 tc.tile_pool(name="pa_w", bufs=2) as pa_w, \
                 tc.tile_pool(name="pa_xt", bufs=1) as pa_xt, \
                 tc.tile_pool(name="pa_ps", bufs=2, space="PSUM") as pa_ps, \
                 tc.tile_pool(name="pa_psv", bufs=2, space="PSUM") as pa_psv, \
                 tc.tile_pool(name="pa_pst", bufs=2, space="PSUM") as pa_pst:

                for slab in range(cfg.slabs):
                    # transpose slab of x; evict as fp16 hi/lo limbs
                    xh = [pa_xt.tile([128, Nq], f16, name=f"xh{c}", tag=f"xh{c}") for c in range(CH)]
                    xl = [pa_xt.tile([128, Nq], f16, name=f"xl{c}", tag=f"xl{c}") for c in range(CH)]
                    for t in range(TQ):
                        xn = pa_sb.tile([128, C], f32, name="xn", tag="xn")
                        nc.sync.dma_start(xn[:], xk_d.ap()[(slab * TQ + t) * 128:(slab * TQ + t + 1) * 128, :])
                        for c in range(CH):
                            pst = pa_pst.tile([128, 128], f32, name="pst", tag="pst")
                            nc.tensor.transpose(pst[:], xn[:, ts(c, 128)], idt_f[:])
                            nc.scalar.copy(xh[c][:, ts(t, 128)], pst[:])
                            nc.vector.tensor_sub(xl[c][:, ts(t, 128)], pst[:], xh[c][:, ts(t, 128)])

                    # K^T limbs (and Q^T limbs on slab 0)
                    for g in range(G):
                        for (w_hi, w_lo, oh, ol, nj, off) in (
                            [(wkh_d, wkl_d, kh_t, kl_t, Nq // J, slab * Nq)] +
                            ([(wqh_d, wql_d, qh_t, ql_t, Nq // J, 0)] if slab == 0 else [])):
                            wgh = pa_w.tile([128, CH, 128], f16, name="wgh", tag="wgh")
                            wgl = pa_w.tile([128, CH, 128], f16, name="wgl", tag="wgl")
                            nc.sync.dma_start(wgh[:], w_hi.ap()[:, ts(g, 128)].rearrange("(c p) n -> p c n", p=128))
                            nc.sync.dma_start(wgl[:], w_lo.ap()[:, ts(g, 128)].rearrange("(c p) n -> p c n", p=128))
                            for j in range(nj):
                                ps = pa_ps.tile([128, J], f32, name="ps_qk", tag="ps_qk")
                                for c in range(CH):
                                    nc.tensor.matmul(ps[:], wgh[:, c, :], xh[c][:, ts(j, J)],
                                                     start=(c == 0), stop=False)
                                    nc.tensor.matmul(ps[:], wgh[:, c, :], xl[c][:, ts(j, J)],
                                                     start=False, stop=False)
                                    nc.tensor.matmul(ps[:], wgl[:, c, :], xh[c][:, ts(j, J)],
                                                     start=False, stop=(c == CH - 1))
                                sl = slice(off + j * J, off + (j + 1) * J)
                                nc.scalar.copy(oh[g][:, sl], ps[:])
                                nc.vector.tensor_sub(ol[g][:, sl], ps[:], oh[g][:, sl])

                    # V (token-major, fp16)
                    for vc_base in range(0, QC, 384):
                        vw = min(384, QC - vc_base)
                        wvg = pa_w.tile([128, CH, 384], f16, name="wvg", tag="wvg")
                        nc.sync.dma_start(
                            wvg[:, :, :vw],
                            wv_d.ap()[:, vc_base:vc_base + vw].rearrange("(c p) n -> p c n", p=128))
                        for t in range(TQ):
                            psv = pa_psv.tile([128, 384], f32, name="psv", tag="psv")
                            for c in range(CH):
                                nc.tensor.matmul(psv[:, :vw], xh[c][:, ts(t, 128)], wvg[:, c, :vw],
                                                 start=(c == 0), stop=(c == CH - 1))
                            nc.vector.tensor_copy(vb[slab * TQ + t][:, vc_base:vc_base + vw], psv[:, :vw])

            # ---------------- Phase B: attention ----------------
            # Blocks of BLK q-tiles; each block's AV is emitted one block
            # late so the PE never stalls on the DMA transposes (HAM stays
            # warm at 2.4GHz).
            BLK = min(4, TQ)
            with tc.tile_pool(name="pb_p", bufs=2) as pb_p, \
                 tc.tile_pool(name="pb_pn", bufs=2) as pb_pn, \
                 tc.tile_pool(name="pb_pth", bufs=3) as pb_pth, \
                 tc.tile_pool(name="pb_st", bufs=3) as pb_st, \
                 tc.tile_pool(name="pb_s", bufs=3, space="PSUM") as pb_s, \
                 tc.tile_pool(name="pb_x1", bufs=2, space="PSUM") as pb_x1:

                def emit_av(g, r, h, qb, pThb):
                    ps_x1 = pb_x1.tile([D, BLK * 128], f32, name="ps_x1", tag="ps_x1")
                    for k in range(TK):
                        nc.tensor.matmul(ps_x1[:],
                                         vb[k][:, h * D:(h + 1) * D],
                                         pThb[:, k, :, :].rearrange("p t q -> p (t q)"),
                                         start=(k == 0), stop=(k == TK - 1))
                    nc.vector.tensor_copy(
                        x1t[g][r:r + D, qb * BLK * 128:(qb + 1) * BLK * 128], ps_x1[:])

                pending = None
                for h in range(H):
                    g, r = divmod(h * D, 128)
                    for qb in range(TQ // BLK):
                        pThb = pb_pth.tile([128, TK, BLK, 128], f16, name="pThb", tag="pThb")
                        for tt in range(BLK):
                            t = qb * BLK + tt
                            qh_s = qh_t[g][r:r + D, ts(t, 128)]
                            ql_s = ql_t[g][r:r + D, ts(t, 128)]
                            s_tiles = []
                            nms = []
                            for sh in range(NSH):
                                ps_s = pb_s.tile([128, SH], f32, name="ps_s", tag="ps_s")
                                for j in range(SH // J):
                                    sl = slice(sh * SH + j * J, sh * SH + (j + 1) * J)
                                    nc.tensor.matmul(ps_s[:, ts(j, J)], qh_s, kh_t[g][r:r + D, sl],
                                                     start=True, stop=False)
                                    nc.tensor.matmul(ps_s[:, ts(j, J)], qh_s, kl_t[g][r:r + D, sl],
                                                     start=False, stop=False)
                                    nc.tensor.matmul(ps_s[:, ts(j, J)], ql_s, kh_t[g][r:r + D, sl],
                                                     start=False, stop=True)
                                nm = pb_st.tile([128, 1], f32, name="nm", tag="nm")
                                nc.vector.reduce_max(out=nm[:], in_=ps_s[:],
                                                     axis=mybir.AxisListType.X, negate=True)
                                s_tiles.append(ps_s)
                                nms.append(nm)
                            negm = pb_st.tile([128, 1], f32, name="negm", tag="negm")
                            if NSH > 1:
                                nc.vector.tensor_tensor(negm[:], nms[0][:], nms[1][:],
                                                        op=mybir.AluOpType.min)
                                for sh in range(2, NSH):
                                    nc.vector.tensor_tensor(negm[:], negm[:], nms[sh][:],
                                                            op=mybir.AluOpType.min)
                            else:
                                nc.vector.tensor_copy(negm[:], nms[0][:])

                            p_t = pb_p.tile([128, Nk], f32, name="p_t", tag="p_t")
                            l_parts = []
                            for sh in range(NSH):
                                l_i = pb_st.tile([128, 1], f32, name=f"l{sh}", tag=f"l{sh}")
                                nc.scalar.activation(p_t[:, sh * SH:(sh + 1) * SH], s_tiles[sh][:],
                                                     mybir.ActivationFunctionType.Exp,
                                                     bias=negm[:], accum_out=l_i[:])
                                l_parts.append(l_i)
                            l_tot = pb_st.tile([128, 1], f32, name="l_tot", tag="l_tot")
                            if NSH > 1:
                                nc.gpsimd.tensor_add(l_tot[:], l_parts[0][:], l_parts[1][:])
                                for sh in range(2, NSH):
                                    nc.gpsimd.tensor_add(l_tot[:], l_tot[:], l_parts[sh][:])
                            else:
                                nc.gpsimd.tensor_copy(l_tot[:], l_parts[0][:])
                            rl = pb_st.tile([128, 1], f32, name="rl", tag="rl")
                            nc.vector.reciprocal(rl[:], l_tot[:])
                            p_n = pb_pn.tile([128, Nk], f16, name="p_n", tag="p_n")
                            nc.vector.tensor_scalar_mul(p_n[:], p_t[:], rl[:])
                            # blockwise transpose: pThb[p, k, tt, q] = p_n[q, k*128+p]
                            nc.sync.dma_start(pThb[:, :, tt, :], p_n[:], transpose=True)

                        if pending is not None:
                            emit_av(*pending)
                        pending = (g, r, h, qb, pThb)
                if pending is not None:
                    emit_av(*pending)

            # ---------------- Phase C: proj + residual + LayerNorm ----------------
            with tc.tile_pool(name="pc_w", bufs=1) as pc_w, \
                 tc.tile_pool(name="pc_sb", bufs=2) as pc_sb, \
                 tc.tile_pool(name="pc_st", bufs=3) as pc_st, \
                 tc.tile_pool(name="pc_ps", bufs=4, space="PSUM") as pc_ps, \
                 tc.tile_pool(name="pc_bc", bufs=3, space="PSUM") as pc_bc:

                # broadcast proj_b / gamma / beta across partitions via ones-matmul
                ones = pc_w.tile([1, 128], f32, name="ones", tag="ones")
                nc.gpsimd.memset(ones[:], 1.0)
                bc = []
                for vi in range(3):
                    vrow = pc_w.tile([1, C], f32, name=f"vrow{vi}", tag=f"vrow{vi}")
                    nc.sync.dma_start(vrow[:], vec_d.ap()[vi:vi + 1, :])
                    bct = pc_w.tile([128, C], f32, name=f"bc{vi}", tag=f"bc{vi}")
                    for j in range(0, C, 512):
                        w = min(512, C - j)
                        psb = pc_bc.tile([128, 512], f32, name="psb", tag="psb")
                        nc.tensor.matmul(psb[:, :w], ones[:], vrow[:, j:j + w],
                                         start=True, stop=True)
                        nc.scalar.copy(bct[:, j:j + w], psb[:, :w])
                    bc.append(bct)
                bias_bc, gam_bc, bet_bc = bc

                wpb = []
                for c in range(G):
                    wpc = pc_w.tile([128, C], f16, name=f"wpb{c}", tag=f"wpb{c}")
                    nc.sync.dma_start(wpc[:], wp_d.ap()[ts(c, 128), :])
                    wpb.append(wpc)

                eps_t = pc_w.tile([128, 1], f32, name="eps_t", tag="eps_t")
                nc.gpsimd.memset(eps_t[:], 1e-5)

                NSTAT = 256
                nsub = C // NSTAT
                for t in range(TQ):
                    pps = []
                    for j in range(0, C, 384):
                        w = min(384, C - j)
                        pp = pc_ps.tile([128, 384], f32, name="pp", tag="pp")
                        for c in range(G):
                            nc.tensor.matmul(pp[:, :w], x1t[c][:, ts(t, 128)], wpb[c][:, j:j + w],
                                             start=(c == 0), stop=(c == G - 1))
                        pps.append((j, w, pp))
                    xr = pc_sb.tile([128, C], f32, name="xr", tag="xr")
                    nc.sync.dma_start(xr[:], xk_d.ap()[ts(t, 128), :])
                    u = pc_sb.tile([128, C], f32, name="u", tag="u")
                    for (j, w, pp) in pps:
                        nc.vector.tensor_add(u[:, j:j + w], pp[:, :w], bias_bc[:, j:j + w])
                    nc.vector.tensor_add(u[:], u[:], xr[:])

                    stats = pc_st.tile([128, nsub, 6], f32, name="stats", tag="stats")
                    for s in range(nsub):
                        nc.vector.bn_stats(out=stats[:, s, :], in_=u[:, ts(s, NSTAT)])
                    mv = pc_st.tile([128, 2], f32, name="mv", tag="mv")
                    nc.vector.bn_aggr(out=mv[:], in_=stats[:])
                    rstd = pc_st.tile([128, 1], f32, name="rstd", tag="rstd")
                    nc.scalar.activation(rstd[:], mv[:, 1:2],
                                         mybir.ActivationFunctionType.Sqrt, bias=eps_t[:])
                    nc.vector.reciprocal(rstd[:], rstd[:])

                    of = pc_sb.tile([128, C], f32, name="of", tag="of")
                    nc.vector.tensor_scalar(out=of[:], in0=u[:], scalar1=mv[:, 0:1],
                                            scalar2=rstd[:], op0=mybir.AluOpType.subtract,
                                            op1=mybir.AluOpType.mult)
                    nc.vector.tensor_mul(of[:], of[:], gam_bc[:])
                    nc.vector.tensor_add(of[:], of[:], bet_bc[:])
                    nc.sync.dma_start(out_d.ap()[ts(t, 128), :], of[:])

    nc.compile()
    return nc


_CACHE = {}


def _get_program(cfg: Cfg):
    if cfg not in _CACHE:
        _CACHE[cfg] = build_program(cfg)
    return _CACHE[cfg]


def _split16(w):
    hi = w.astype(np.float16)
    lo = (w - hi.astype(np.float32)).astype(np.float16)
    return np.ascontiguousarray(hi), np.ascontiguousarray(lo)


def make_in_maps(x, qkv_w, proj_w, proj_b, ln_gamma, ln_beta, cfg: Cfg):
    """Host-side shard prep. Returns list of 8 in_maps."""
    C = cfg.C
    B = x.shape[0]
    wq_t = np.ascontiguousarray((qkv_w[0:C] * np.float32(cfg.D ** 0.5)).T)
    wk_t = np.ascontiguousarray(qkv_w[C:2 * C].T)
    wv_t = np.ascontiguousarray(qkv_w[2 * C:3 * C].T)
    wp_t = np.ascontiguousarray(proj_w.T)
    wq_hi, wq_lo = _split16(wq_t)
    wk_hi, wk_lo = _split16(wk_t)
    wv_h = wv_t.astype(np.float16)
    wp_h = wp_t.astype(np.float16)
    vecs = np.ascontiguousarray(np.stack([proj_b, ln_gamma, ln_beta]).astype(np.float32))
    in_maps = []
    for core in range(8):
        b, half = core // 2, core % 2
        b = min(b, B - 1)
        xb = np.asarray(x[b], dtype=np.float32)
        if half == 0:
            xkc = np.ascontiguousarray(xb)
        else:
            xkc = np.ascontiguousarray(np.concatenate([xb[cfg.Nq:], xb[:cfg.Nq]], axis=0))
        in_maps.append({"xk": xkc, "wq_hi": wq_hi, "wq_lo": wq_lo,
                        "wk_hi": wk_hi, "wk_lo": wk_lo, "wv_h": wv_h,
                        "wp_h": wp_h, "vecs": vecs})
    return in_maps


def kernel(x, qkv_w, proj_w, proj_b, ln_gamma, ln_beta):
    from concourse.bass_utils import run_bass_kernel_spmd

    cfg = Cfg()
    nc = _get_program(cfg)
    x = np.asarray(x, dtype=np.float32)
    in_maps = make_in_maps(x, np.asarray(qkv_w, np.float32), np.asarray(proj_w, np.float32),
                           np.asarray(proj_b, np.float32), np.asarray(ln_gamma, np.float32),
                           np.asarray(ln_beta, np.float32), cfg)
    res = run_bass_kernel_spmd(nc, in_maps, core_ids=list(range(8)))
    B, N, C = x.shape
    out = np.empty((B, N, C), dtype=np.float32)
    for core in range(8):
        b, half = core // 2, core % 2
        out[b, half * cfg.Nq:(half + 1) * cfg.Nq] = res.results[core]["out"]
    return out


# revision 21
# speedup vs baseline: 3.9210x; 1.1596x over previous
"""Trainium2 Bass kernel for nn_ECA (attention block + residual + LayerNorm).

Reference computation (per batch b):
    qkv = x @ qkv_w.T ; q,k,v per head
    attn = softmax((q @ k.T) * sqrt(D))
    x1 = attn @ v  -> concat heads -> @ proj_w.T + proj_b
    out = LayerNorm(x + x1) * gamma + beta     # eps 1e-5

Sharding: 8 cores = 4 batches x 2 query-halves. Each core receives the full
batch's tokens ("xk", rolled so its own 1024 query tokens are rows 0:1024),
computes K/V for all 2048 keys (duplicated across the 2 cores of a batch),
attention + proj + LN for its 1024 queries. No collectives.

Precision: logits need fp32-class accuracy (softmax here is near-argmax:
logit std ~64, top-2 gap ~13 -- tf32/bf16/fp16 single-pass rounding flips
argmaxes). Native fp32 matmul runs at 1/4 rate with poor pipelining, so Q/K/S
use fp16 limb-split matmuls instead: a = ah + al (fp16 high/low limbs),
a.b = ah.bh + ah.bl + al.bh (3 full-rate fp16 passes, ~2e-5 logit error).
V / P / attn@V / proj run in plain fp16 (smooth ~0.1% errors).
The sqrt(D)=8 scale is folded into wq on the host (exact, power of 2).
"""

import sys
from dataclasses import dataclass

import numpy as np

try:
    import concourse.bass  # noqa: F401
except ImportError:  # fresh dir without sitecustomize path
    sys.path.insert(0, "/opt/trn_rl_repo")


@dataclass(frozen=True)
class Cfg:
    Nk: int = 2048   # keys per core (full batch)
    Nq: int = 1024   # queries per core
    C: int = 768     # model dim (also total head dim H*D)
    H: int = 12
    D: int = 64

    @property
    def CH(self):
        return self.C // 128

    @property
    def G(self):
        return (self.H * self.D) // 128

    @property
    def TQ(self):
        return self.Nq // 128

    @property
    def TK(self):
        return self.Nk // 128

    @property
    def slabs(self):
        return self.Nk // self.Nq


def build_program(cfg: Cfg):
    import concourse.bass as bass
    import concourse.mybir as mybir
    import concourse.tile as tile
    from concourse import bacc
    from concourse.masks import make_identity

    f32 = mybir.dt.float32
    f16 = mybir.dt.float16
    ts = bass.ts
    Nk, Nq, C, H, D = cfg.Nk, cfg.Nq, cfg.C, cfg.H, cfg.D
    CH, G, TQ, TK = cfg.CH, cfg.G, cfg.TQ, cfg.TK
    QC = H * D
    assert QC % 128 == 0 and C % 128 == 0 and Nq % 128 == 0

    nc = bacc.Bacc("TRN2", target_bir_lowering=False, debug=False, num_devices=8)

    xk_d = nc.dram_tensor("xk", [Nk, C], f32, kind="ExternalInput")
    wqh_d = nc.dram_tensor("wq_hi", [C, QC], f16, kind="ExternalInput")
    wql_d = nc.dram_tensor("wq_lo", [C, QC], f16, kind="ExternalInput")
    wkh_d = nc.dram_tensor("wk_hi", [C, QC], f16, kind="ExternalInput")
    wkl_d = nc.dram_tensor("wk_lo", [C, QC], f16, kind="ExternalInput")
    wv_d = nc.dram_tensor("wv_h", [C, QC], f16, kind="ExternalInput")
    wp_d = nc.dram_tensor("wp_h", [QC, C], f16, kind="ExternalInput")
    vec_d = nc.dram_tensor("vecs", [3, C], f32, kind="ExternalInput")
    out_d = nc.dram_tensor("out", [Nq, C], f32, kind="ExternalOutput")

    SH = min(Nk, 1024)     # S psum half size
    NSH = Nk // SH
    J = min(512, Nq)       # psum free chunk
    JB = min(512, Nq)      # AV free chunk

    with tile.TileContext(nc) as tc:
        with tc.tile_pool(name="persist", bufs=1) as persist:
            idt_f = persist.tile([128, 128], f32, name="idt_f", tag="idt_f")
            make_identity(nc, idt_f[:])

            kh_t = [persist.tile([128, Nk], f16, name=f"kh{g}", tag=f"kh{g}") for g in range(G)]
            kl_t = [persist.tile([128, Nk], f16, name=f"kl{g}", tag=f"kl{g}") for g in range(G)]
            qh_t = [persist.tile([128, Nq], f16, name=f"qh{g}", tag=f"qh{g}") for g in range(G)]
            ql_t = [persist.tile([128, Nq], f16, name=f"ql{g}", tag=f"ql{g}") for g in range(G)]
            vb = [persist.tile([128, QC], f16, name=f"vb{t}", tag=f"vb{t}") for t in range(TK)]
            x1t = [persist.tile([128, Nq], f16, name=f"x1t{g}", tag=f"x1t{g}") for g in range(G)]

            # ---------------- Phase A: x^T limbs, Q^T, K^T, V ----------------
            with tc.tile_pool(name="pa_sb", bufs=2) as pa_sb, \
                 tc.tile_pool(name="pa_w", bufs=2) as pa_w, \
                 tc.tile_pool(name="pa_xt", bufs=1) as pa_xt, \
                 tc.tile_pool(name="pa_ps", bufs=2, space="PSUM") as pa_ps, \
                 tc.tile_pool(name="pa_psv", bufs=2, space="PSUM") as pa_psv, \
                 tc.tile_pool(name="pa_pst", bufs=2, space="PSUM") as pa_pst:

                for slab in range(cfg.slabs):
                    # transpose slab of x; evict as fp16 hi/lo limbs
                    xh = [pa_xt.tile([128, Nq], f16, name=f"xh{c}", tag=f"xh{c}") for c in range(CH)]
                    xl = [pa_xt.tile([128, Nq], f16, name=f"xl{c}", tag=f"xl{c}") for c in range(CH)]
                    for t in range(TQ):
                        xn = pa_sb.tile([128, C], f32, name="xn", tag="xn")
                        nc.sync.dma_start(xn[:], xk_d.ap()[(slab * TQ + t) * 128:(slab * TQ + t + 1) * 128, :])
                        for c in range(CH):
                            pst = pa_pst.tile([128, 128], f32, name="pst", tag="pst")
                            nc.tensor.transpose(pst[:], xn[:, ts(c, 128)], idt_f[:])
                            nc.scalar.copy(xh[c][:, ts(t, 128)], pst[:])
                            nc.vector.tensor_sub(xl[c][:, ts(t, 128)], pst[:], xh[c][:, ts(t, 128)])

                    # K^T limbs (and Q^T limbs on slab 0)
                    for g in range(G):
                        for (w_hi, w_lo, oh, ol, nj, off) in (
                            [(wkh_d, wkl_d, kh_t, kl_t, Nq // J, slab * Nq)] +
                            ([(wqh_d, wql_d, qh_t, ql_t, Nq // J, 0)] if slab == 0 else [])):
                            wgh = pa_w.tile([128, CH, 128], f16, name="wgh", tag="wgh")
                            wgl = pa_w.tile([128, CH, 128], f16, name="wgl", tag="wgl")
                            nc.sync.dma_start(wgh[:], w_hi.ap()[:, ts(g, 128)].rearrange("(c p) n -> p c n", p=128))
                            nc.sync.dma_start(wgl[:], w_lo.ap()[:, ts(g, 128)].rearrange("(c p) n -> p c n", p=128))
                            for j in range(nj):
                                ps = pa_ps.tile([128, J], f32, name="ps_qk", tag="ps_qk")
                                for c in range(CH):
                                    nc.tensor.matmul(ps[:], wgh[:, c, :], xh[c][:, ts(j, J)],
                                                     start=(c == 0), stop=False)
                                    nc.tensor.matmul(ps[:], wgh[:, c, :], xl[c][:, ts(j, J)],
                                                     start=False, stop=False)
                                    nc.tensor.matmul(ps[:], wgl[:, c, :], xh[c][:, ts(j, J)],
                                                     start=False, stop=(c == CH - 1))
                                sl = slice(off + j * J, off + (j + 1) * J)
                                nc.scalar.copy(oh[g][:, sl], ps[:])
                                nc.vector.tensor_sub(ol[g][:, sl], ps[:], oh[g][:, sl])

                    # V (token-major, fp16)
                    for vc_base in range(0, QC, 384):
                        vw = min(384, QC - vc_base)
                        wvg = pa_w.tile([128, CH, 384], f16, name="wvg", tag="wvg")
                        nc.sync.dma_start(
                            wvg[:, :, :vw],
                            wv_d.ap()[:, vc_base:vc_base + vw].rearrange("(c p) n -> p c n", p=128))
                        for t in range(TQ):
                            psv = pa_psv.tile([128, 384], f32, name="psv", tag="psv")
                            for c in range(CH):
                                nc.tensor.matmul(psv[:, :vw], xh[c][:, ts(t, 128)], wvg[:, c, :vw],
                                                 start=(c == 0), stop=(c == CH - 1))
                            nc.vector.tensor_copy(vb[slab * TQ + t][:, vc_base:vc_base + vw], psv[:, :vw])

            # ---------------- Phase B: attention ----------------
            # Blocks of BLK q-tiles; each block's AV is emitted one block
            # late so the PE never stalls on the DMA transposes (HAM stays
            # warm at 2.4GHz).
            BLK = min(4, TQ)
            JQ = min(512, Nk)   # S psum quarter size
            NQS = Nk // JQ
            with tc.tile_pool(name="pb_p", bufs=2) as pb_p, \
                 tc.tile_pool(name="pb_pn", bufs=2) as pb_pn, \
                 tc.tile_pool(name="pb_pth", bufs=3) as pb_pth, \
                 tc.tile_pool(name="pb_st", bufs=3) as pb_st, \
                 tc.tile_pool(name="pb_s", bufs=5, space="PSUM") as pb_s, \
                 tc.tile_pool(name="pb_x1", bufs=2, space="PSUM") as pb_x1:

                def emit_av(g, r, h, qb, pThb):
                    ps_x1 = pb_x1.tile([D, BLK * 128], f32, name="ps_x1", tag="ps_x1")
                    for k in range(TK):
                        nc.tensor.matmul(ps_x1[:],
                                         vb[k][:, h * D:(h + 1) * D],
                                         pThb[:, k, :, :].rearrange("p t q -> p (t q)"),
                                         start=(k == 0), stop=(k == TK - 1))
                    nc.vector.tensor_copy(
                        x1t[g][r:r + D, qb * BLK * 128:(qb + 1) * BLK * 128], ps_x1[:])

                pending = None
                for h in range(H):
                    g, r = divmod(h * D, 128)
                    for qb in range(TQ // BLK):
                        pThb = pb_pth.tile([128, TK, BLK, 128], f16, name="pThb", tag="pThb")
                        for tt in range(BLK):
                            t = qb * BLK + tt
                            qh_s = qh_t[g][r:r + D, ts(t, 128)]
                            ql_s = ql_t[g][r:r + D, ts(t, 128)]
                            # quarterwise online softmax: each JQ-key quarter
                            # gets its own local max + exp (values <= 1, no
                            # overflow, no cross-quarter serialization); tiny
                            # (128,NQS) fixups rescale by exp(m_j - m) / l.
                            p_t = pb_p.tile([128, Nk], f32, name="p_t", tag="p_t")
                            nm_pack = pb_st.tile([128, NQS], f32, name="nm_pack", tag="nm_pack")
                            l_pack = pb_st.tile([128, NQS], f32, name="l_pack", tag="l_pack")
                            for j4 in range(NQS):
                                ps_s = pb_s.tile([128, JQ], f32, name="ps_s", tag="ps_s")
                                sl = slice(j4 * JQ, (j4 + 1) * JQ)
                                nc.tensor.matmul(ps_s[:], qh_s, kh_t[g][r:r + D, sl],
                                                 start=True, stop=False)
                                nc.tensor.matmul(ps_s[:], qh_s, kl_t[g][r:r + D, sl],
                                                 start=False, stop=False)
                                nc.tensor.matmul(ps_s[:], ql_s, kh_t[g][r:r + D, sl],
                                                 start=False, stop=True)
                                nc.vector.reduce_max(out=nm_pack[:, j4:j4 + 1], in_=ps_s[:],
                                                     axis=mybir.AxisListType.X, negate=True)
                                nc.scalar.activation(p_t[:, sl], ps_s[:],
                                                     mybir.ActivationFunctionType.Exp,
                                                     bias=nm_pack[:, j4:j4 + 1],
                                                     accum_out=l_pack[:, j4:j4 + 1])
                            # fixups: negm = min_j nm_j (= -m); d = nm_j-negm = m-m_j
                            # e_j = exp(-d); l = sum l_j e_j ; sc_j = e_j / l
                            negm = pb_st.tile([128, 1], f32, name="negm", tag="negm")
                            sc = pb_st.tile([128, NQS], f32, name="sc", tag="sc")
                            if NQS > 1:
                                nc.vector.tensor_reduce(out=negm[:], in_=nm_pack[:],
                                                        axis=mybir.AxisListType.X,
                                                        op=mybir.AluOpType.min)
                                d_p = pb_st.tile([128, NQS], f32, name="d_p", tag="d_p")
                                nc.vector.tensor_scalar(out=d_p[:], in0=nm_pack[:],
                                                        scalar1=negm[:], scalar2=None,
                                                        op0=mybir.AluOpType.subtract)
                                e_p = pb_st.tile([128, NQS], f32, name="e_p", tag="e_p")
                                nc.scalar.activation(e_p[:], d_p[:],
                                                     mybir.ActivationFunctionType.Exp,
                                                     scale=-1.0)
                                lw = pb_st.tile([128, NQS], f32, name="lw", tag="lw")
                                nc.vector.tensor_mul(lw[:], l_pack[:], e_p[:])
                                l_tot = pb_st.tile([128, 1], f32, name="l_tot", tag="l_tot")
                                nc.vector.reduce_sum(out=l_tot[:], in_=lw[:],
                                                     axis=mybir.AxisListType.X)
                                rl = pb_st.tile([128, 1], f32, name="rl", tag="rl")
                                nc.vector.reciprocal(rl[:], l_tot[:])
                                nc.vector.tensor_scalar_mul(sc[:], e_p[:], rl[:])
                            else:
                                nc.vector.reciprocal(sc[:], l_pack[:])
                            p_n = pb_pn.tile([128, Nk], f16, name="p_n", tag="p_n")
                            for j4 in range(NQS):
                                nc.vector.tensor_scalar_mul(p_n[:, ts(j4, JQ)],
                                                            p_t[:, ts(j4, JQ)],
                                                            sc[:, j4:j4 + 1])
                            # blockwise transpose: pThb[p, k, tt, q] = p_n[q, k*128+p]
                            nc.sync.dma_start(pThb[:, :, tt, :], p_n[:], transpose=True)

                        if pending is not None:
                            emit_av(*pending)
                        pending = (g, r, h, qb, pThb)
                if pending is not None:
                    emit_av(*pending)

            # ---------------- Phase C: proj + residual + LayerNorm ----------------
            with tc.tile_pool(name="pc_w", bufs=1) as pc_w, \
                 tc.tile_pool(name="pc_sb", bufs=2) as pc_sb, \
                 tc.tile_pool(name="pc_st", bufs=3) as pc_st, \
                 tc.tile_pool(name="pc_ps", bufs=4, space="PSUM") as pc_ps, \
                 tc.tile_pool(name="pc_bc", bufs=3, space="PSUM") as pc_bc:

                # broadcast proj_b / gamma / beta across partitions via ones-matmul
                ones = pc_w.tile([1, 128], f32, name="ones", tag="ones")
                nc.gpsimd.memset(ones[:], 1.0)
                bc = []
                for vi in range(3):
                    vrow = pc_w.tile([1, C], f32, name=f"vrow{vi}", tag=f"vrow{vi}")
                    nc.sync.dma_start(vrow[:], vec_d.ap()[vi:vi + 1, :])
                    bct = pc_w.tile([128, C], f32, name=f"bc{vi}", tag=f"bc{vi}")
                    for j in range(0, C, 512):
                        w = min(512, C - j)
                        psb = pc_bc.tile([128, 512], f32, name="psb", tag="psb")
                        nc.tensor.matmul(psb[:, :w], ones[:], vrow[:, j:j + w],
                                         start=True, stop=True)
                        nc.scalar.copy(bct[:, j:j + w], psb[:, :w])
                    bc.append(bct)
                bias_bc, gam_bc, bet_bc = bc

                wpb = []
                for c in range(G):
                    wpc = pc_w.tile([128, C], f16, name=f"wpb{c}", tag=f"wpb{c}")
                    nc.sync.dma_start(wpc[:], wp_d.ap()[ts(c, 128), :])
                    wpb.append(wpc)

                eps_t = pc_w.tile([128, 1], f32, name="eps_t", tag="eps_t")
                nc.gpsimd.memset(eps_t[:], 1e-5)

                NSTAT = 256
                nsub = C // NSTAT
                for t in range(TQ):
                    pps = []
                    for j in range(0, C, 384):
                        w = min(384, C - j)
                        pp = pc_ps.tile([128, 384], f32, name="pp", tag="pp")
                        for c in range(G):
                            nc.tensor.matmul(pp[:, :w], x1t[c][:, ts(t, 128)], wpb[c][:, j:j + w],
                                             start=(c == 0), stop=(c == G - 1))
                        pps.append((j, w, pp))
                    xr = pc_sb.tile([128, C], f32, name="xr", tag="xr")
                    nc.sync.dma_start(xr[:], xk_d.ap()[ts(t, 128), :])
                    u = pc_sb.tile([128, C], f32, name="u", tag="u")
                    for (j, w, pp) in pps:
                        nc.vector.tensor_add(u[:, j:j + w], pp[:, :w], bias_bc[:, j:j + w])
                    nc.vector.tensor_add(u[:], u[:], xr[:])

                    stats = pc_st.tile([128, nsub, 6], f32, name="stats", tag="stats")
                    for s in range(nsub):
                        nc.vector.bn_stats(out=stats[:, s, :], in_=u[:, ts(s, NSTAT)])
                    mv = pc_st.tile([128, 2], f32, name="mv", tag="mv")
                    nc.vector.bn_aggr(out=mv[:], in_=stats[:])
                    rstd = pc_st.tile([128, 1], f32, name="rstd", tag="rstd")
                    nc.scalar.activation(rstd[:], mv[:, 1:2],
                                         mybir.ActivationFunctionType.Sqrt, bias=eps_t[:])
                    nc.vector.reciprocal(rstd[:], rstd[:])

                    of = pc_sb.tile([128, C], f32, name="of", tag="of")
                    nc.vector.tensor_scalar(out=of[:], in0=u[:], scalar1=mv[:, 0:1],
                                            scalar2=rstd[:], op0=mybir.AluOpType.subtract,
                                            op1=mybir.AluOpType.mult)
                    nc.vector.tensor_mul(of[:], of[:], gam_bc[:])
                    nc.vector.tensor_add(of[:], of[:], bet_bc[:])
                    nc.sync.dma_start(out_d.ap()[ts(t, 128), :], of[:])

    nc.compile()
    return nc


_CACHE = {}


def _get_program(cfg: Cfg):
    if cfg not in _CACHE:
        _CACHE[cfg] = build_program(cfg)
    return _CACHE[cfg]


def _split16(w):
    hi = w.astype(np.float16)
    lo = (w - hi.astype(np.float32)).astype(np.float16)
    return np.ascontiguousarray(hi), np.ascontiguousarray(lo)


def make_in_maps(x, qkv_w, proj_w, proj_b, ln_gamma, ln_beta, cfg: Cfg):
    """Host-side shard prep. Returns list of 8 in_maps."""
    C = cfg.C
    B = x.shape[0]
    wq_t = np.ascontiguousarray((qkv_w[0:C] * np.float32(cfg.D ** 0.5)).T)
    wk_t = np.ascontiguousarray(qkv_w[C:2 * C].T)
    wv_t = np.ascontiguousarray(qkv_w[2 * C:3 * C].T)
    wp_t = np.ascontiguousarray(proj_w.T)
    wq_hi, wq_lo = _split16(wq_t)
    wk_hi, wk_lo = _split16(wk_t)
    wv_h = wv_t.astype(np.float16)
    wp_h = wp_t.astype(np.float16)
    vecs = np.ascontiguousarray(np.stack([proj_b, ln_gamma, ln_beta]).astype(np.float32))
    in_maps = []
    for core in range(8):
        b, half = core // 2, core % 2
        b = min(b, B - 1)
        xb = np.asarray(x[b], dtype=np.float32)
        if half == 0:
            xkc = np.ascontiguousarray(xb)
        else:
            xkc = np.ascontiguousarray(np.concatenate([xb[cfg.Nq:], xb[:cfg.Nq]], axis=0))
        in_maps.append({"xk": xkc, "wq_hi": wq_hi, "wq_lo": wq_lo,
                        "wk_hi": wk_hi, "wk_lo": wk_lo, "wv_h": wv_h,
                        "wp_h": wp_h, "vecs": vecs})
    return in_maps


def kernel(x, qkv_w, proj_w, proj_b, ln_gamma, ln_beta):
    from concourse.bass_utils import run_bass_kernel_spmd

    cfg = Cfg()
    nc = _get_program(cfg)
    x = np.asarray(x, dtype=np.float32)
    in_maps = make_in_maps(x, np.asarray(qkv_w, np.float32), np.asarray(proj_w, np.float32),
                           np.asarray(proj_b, np.float32), np.asarray(ln_gamma, np.float32),
                           np.asarray(ln_beta, np.float32), cfg)
    res = run_bass_kernel_spmd(nc, in_maps, core_ids=list(range(8)))
    B, N, C = x.shape
    out = np.empty((B, N, C), dtype=np.float32)
    for core in range(8):
        b, half = core // 2, core % 2
        out[b, half * cfg.Nq:(half + 1) * cfg.Nq] = res.results[core]["out"]
    return out


# revision 27
# speedup vs baseline: 3.9603x; 1.0100x over previous
"""Trainium2 Bass kernel for nn_ECA (attention block + residual + LayerNorm).

Reference computation (per batch b):
    qkv = x @ qkv_w.T ; q,k,v per head
    attn = softmax((q @ k.T) * sqrt(D))
    x1 = attn @ v  -> concat heads -> @ proj_w.T + proj_b
    out = LayerNorm(x + x1) * gamma + beta     # eps 1e-5

Sharding: 8 cores = 4 batches x 2 query-halves. Each core receives the full
batch's tokens ("xk", rolled so its own 1024 query tokens are rows 0:1024),
computes K/V for all 2048 keys (duplicated across the 2 cores of a batch),
attention + proj + LN for its 1024 queries. No collectives.

Precision: logits need fp32-class accuracy (softmax here is near-argmax:
logit std ~64, top-2 gap ~13 -- tf32/bf16/fp16 single-pass rounding flips
argmaxes). Native fp32 matmul runs at 1/4 rate with poor pipelining, so Q/K/S
use fp16 limb-split matmuls instead: a = ah + al (fp16 high/low limbs),
a.b = ah.bh + ah.bl + al.bh (3 full-rate fp16 passes, ~2e-5 logit error).
V / P / attn@V / proj run in plain fp16 (smooth ~0.1% errors).
The sqrt(D)=8 scale is folded into wq on the host (exact, power of 2).
"""

import sys
from dataclasses import dataclass

import numpy as np

try:
    import concourse.bass  # noqa: F401
except ImportError:  # fresh dir without sitecustomize path
    sys.path.insert(0, "/opt/trn_rl_repo")


@dataclass(frozen=True)
class Cfg:
    Nk: int = 2048   # keys per core (full batch)
    Nq: int = 1024   # queries per core
    C: int = 768     # model dim (also total head dim H*D)
    H: int = 12
    D: int = 64

    @property
    def CH(self):
        return self.C // 128

    @property
    def G(self):
        return (self.H * self.D) // 128

    @property
    def TQ(self):
        return self.Nq // 128

    @property
    def TK(self):
        return self.Nk // 128

    @property
    def slabs(self):
        return self.Nk // self.Nq


def build_program(cfg: Cfg):
    import concourse.bass as bass
    import concourse.mybir as mybir
    import concourse.tile as tile
    from concourse import bacc
    from concourse.masks import make_identity

    f32 = mybir.dt.float32
    f16 = mybir.dt.float16
    ts = bass.ts
    Nk, Nq, C, H, D = cfg.Nk, cfg.Nq, cfg.C, cfg.H, cfg.D
    CH, G, TQ, TK = cfg.CH, cfg.G, cfg.TQ, cfg.TK
    QC = H * D
    assert QC % 128 == 0 and C % 128 == 0 and Nq % 128 == 0

    nc = bacc.Bacc("TRN2", target_bir_lowering=False, debug=False, num_devices=8)

    xk_d = nc.dram_tensor("xk", [Nk, C], f32, kind="ExternalInput")
    wqh_d = nc.dram_tensor("wq_hi", [C, QC], f16, kind="ExternalInput")
    wql_d = nc.dram_tensor("wq_lo", [C, QC], f16, kind="ExternalInput")
    wkh_d = nc.dram_tensor("wk_hi", [C, QC], f16, kind="ExternalInput")
    wkl_d = nc.dram_tensor("wk_lo", [C, QC], f16, kind="ExternalInput")
    wv_d = nc.dram_tensor("wv_h", [C, QC], f16, kind="ExternalInput")
    wp_d = nc.dram_tensor("wp_h", [QC, C], f16, kind="ExternalInput")
    vec_d = nc.dram_tensor("vecs", [3, C], f32, kind="ExternalInput")
    out_d = nc.dram_tensor("out", [Nq, C], f32, kind="ExternalOutput")

    SH = min(Nk, 1024)     # S psum half size
    NSH = Nk // SH
    J = min(512, Nq)       # psum free chunk
    JB = min(512, Nq)      # AV free chunk

    with tile.TileContext(nc) as tc:
        with tc.tile_pool(name="persist", bufs=1) as persist:
            idt_f = persist.tile([128, 128], f32, name="idt_f", tag="idt_f")
            make_identity(nc, idt_f[:])

            kh_t = [persist.tile([128, Nk], f16, name=f"kh{g}", tag=f"kh{g}") for g in range(G)]
            kl_t = [persist.tile([128, Nk], f16, name=f"kl{g}", tag=f"kl{g}") for g in range(G)]
            qh_t = [persist.tile([128, Nq], f16, name=f"qh{g}", tag=f"qh{g}") for g in range(G)]
            ql_t = [persist.tile([128, Nq], f16, name=f"ql{g}", tag=f"ql{g}") for g in range(G)]
            vb = [persist.tile([128, QC], f16, name=f"vb{t}", tag=f"vb{t}") for t in range(TK)]
            x1t = [persist.tile([128, Nq], f16, name=f"x1t{g}", tag=f"x1t{g}") for g in range(G)]

            # ---------------- Phase A: x^T limbs, Q^T, K^T, V ----------------
            with tc.tile_pool(name="pa_sb", bufs=2) as pa_sb, \
                 tc.tile_pool(name="pa_w", bufs=2) as pa_w, \
                 tc.tile_pool(name="pa_xt", bufs=1) as pa_xt, \
                 tc.tile_pool(name="pa_ps", bufs=2, space="PSUM") as pa_ps, \
                 tc.tile_pool(name="pa_psv", bufs=2, space="PSUM") as pa_psv, \
                 tc.tile_pool(name="pa_pst", bufs=2, space="PSUM") as pa_pst:

                for slab in range(cfg.slabs):
                    # transpose slab of x; evict as fp16 hi/lo limbs
                    xh = [pa_xt.tile([128, Nq], f16, name=f"xh{c}", tag=f"xh{c}") for c in range(CH)]
                    xl = [pa_xt.tile([128, Nq], f16, name=f"xl{c}", tag=f"xl{c}") for c in range(CH)]
                    for t in range(TQ):
                        xn = pa_sb.tile([128, C], f32, name="xn", tag="xn")
                        nc.sync.dma_start(xn[:], xk_d.ap()[(slab * TQ + t) * 128:(slab * TQ + t + 1) * 128, :])
                        for c in range(CH):
                            pst = pa_pst.tile([128, 128], f32, name="pst", tag="pst")
                            nc.tensor.transpose(pst[:], xn[:, ts(c, 128)], idt_f[:])
                            nc.scalar.copy(xh[c][:, ts(t, 128)], pst[:])
                            nc.vector.tensor_sub(xl[c][:, ts(t, 128)], pst[:], xh[c][:, ts(t, 128)])

                    # K^T limbs (and Q^T limbs on slab 0)
                    for g in range(G):
                        for (w_hi, w_lo, oh, ol, nj, off) in (
                            [(wkh_d, wkl_d, kh_t, kl_t, Nq // J, slab * Nq)] +
                            ([(wqh_d, wql_d, qh_t, ql_t, Nq // J, 0)] if slab == 0 else [])):
                            wgh = pa_w.tile([128, CH, 128], f16, name="wgh", tag="wgh")
                            wgl = pa_w.tile([128, CH, 128], f16, name="wgl", tag="wgl")
                            nc.sync.dma_start(wgh[:], w_hi.ap()[:, ts(g, 128)].rearrange("(c p) n -> p c n", p=128))
                            nc.sync.dma_start(wgl[:], w_lo.ap()[:, ts(g, 128)].rearrange("(c p) n -> p c n", p=128))
                            for j in range(nj):
                                ps = pa_ps.tile([128, J], f32, name="ps_qk", tag="ps_qk")
                                for c in range(CH):
                                    nc.tensor.matmul(ps[:], wgh[:, c, :], xh[c][:, ts(j, J)],
                                                     start=(c == 0), stop=False)
                                    nc.tensor.matmul(ps[:], wgh[:, c, :], xl[c][:, ts(j, J)],
                                                     start=False, stop=False)
                                    nc.tensor.matmul(ps[:], wgl[:, c, :], xh[c][:, ts(j, J)],
                                                     start=False, stop=(c == CH - 1))
                                sl = slice(off + j * J, off + (j + 1) * J)
                                nc.scalar.copy(oh[g][:, sl], ps[:])
                                nc.vector.tensor_sub(ol[g][:, sl], ps[:], oh[g][:, sl])

                    # V (token-major, fp16)
                    for vc_base in range(0, QC, 384):
                        vw = min(384, QC - vc_base)
                        wvg = pa_w.tile([128, CH, 384], f16, name="wvg", tag="wvg")
                        nc.sync.dma_start(
                            wvg[:, :, :vw],
                            wv_d.ap()[:, vc_base:vc_base + vw].rearrange("(c p) n -> p c n", p=128))
                        for t in range(TQ):
                            psv = pa_psv.tile([128, 384], f32, name="psv", tag="psv")
                            for c in range(CH):
                                nc.tensor.matmul(psv[:, :vw], xh[c][:, ts(t, 128)], wvg[:, c, :vw],
                                                 start=(c == 0), stop=(c == CH - 1))
                            nc.vector.tensor_copy(vb[slab * TQ + t][:, vc_base:vc_base + vw], psv[:, :vw])

            # ---------------- Phase B: attention ----------------
            # Blocks of BLK q-tiles; each block's AV is emitted one block
            # late so the PE never stalls on the DMA transposes (HAM stays
            # warm at 2.4GHz).
            BLK = min(4, TQ)
            JQ = min(512, Nk)   # S psum slice size (local-max granularity)
            NQS = Nk // JQ
            with tc.tile_pool(name="pb_p", bufs=2) as pb_p, \
                 tc.tile_pool(name="pb_pn", bufs=2) as pb_pn, \
                 tc.tile_pool(name="pb_pth", bufs=3) as pb_pth, \
                 tc.tile_pool(name="pb_st", bufs=3) as pb_st, \
                 tc.tile_pool(name="pb_s", bufs=5, space="PSUM") as pb_s, \
                 tc.tile_pool(name="pb_x1", bufs=2, space="PSUM") as pb_x1:

                def emit_av(g, r, h, qb, pThb):
                    ps_x1 = pb_x1.tile([D, BLK * 128], f32, name="ps_x1", tag="ps_x1")
                    for k in range(TK):
                        nc.tensor.matmul(ps_x1[:],
                                         vb[k][:, h * D:(h + 1) * D],
                                         pThb[:, k, :, :].rearrange("p t q -> p (t q)"),
                                         start=(k == 0), stop=(k == TK - 1))
                    nc.vector.tensor_copy(
                        x1t[g][r:r + D, qb * BLK * 128:(qb + 1) * BLK * 128], ps_x1[:])

                pending = None
                for h in range(H):
                    g, r = divmod(h * D, 128)
                    for qb in range(TQ // BLK):
                        pThb = pb_pth.tile([128, TK, BLK, 128], f16, name="pThb", tag="pThb")
                        for tt in range(BLK):
                            t = qb * BLK + tt
                            qh_s = qh_t[g][r:r + D, ts(t, 128)]
                            ql_s = ql_t[g][r:r + D, ts(t, 128)]
                            # quarterwise online softmax: each JQ-key quarter
                            # gets its own local max + exp (values <= 1, no
                            # overflow, no cross-quarter serialization); tiny
                            # (128,NQS) fixups rescale by exp(m_j - m) / l.
                            p_t = pb_p.tile([128, Nk], f32, name="p_t", tag="p_t")
                            nm_pack = pb_st.tile([128, NQS], f32, name="nm_pack", tag="nm_pack")
                            l_pack = pb_st.tile([128, NQS], f32, name="l_pack", tag="l_pack")
                            for j4 in range(NQS):
                                ps_s = pb_s.tile([128, JQ], f32, name="ps_s", tag="ps_s")
                                sl = slice(j4 * JQ, (j4 + 1) * JQ)
                                for j in range(JQ // J):
                                    sj = slice(j4 * JQ + j * J, j4 * JQ + (j + 1) * J)
                                    nc.tensor.matmul(ps_s[:, ts(j, J)], qh_s, kh_t[g][r:r + D, sj],
                                                     start=True, stop=False)
                                    nc.tensor.matmul(ps_s[:, ts(j, J)], qh_s, kl_t[g][r:r + D, sj],
                                                     start=False, stop=False)
                                    nc.tensor.matmul(ps_s[:, ts(j, J)], ql_s, kh_t[g][r:r + D, sj],
                                                     start=False, stop=True)
                                nc.vector.reduce_max(out=nm_pack[:, j4:j4 + 1], in_=ps_s[:],
                                                     axis=mybir.AxisListType.X, negate=True)
                                nc.scalar.activation(p_t[:, sl], ps_s[:],
                                                     mybir.ActivationFunctionType.Exp,
                                                     bias=nm_pack[:, j4:j4 + 1],
                                                     accum_out=l_pack[:, j4:j4 + 1])
                            # fixups: negm = min_j nm_j (= -m); e_j = exp(m_j - m)
                            # l = sum_j l_j e_j ; sc_j = e_j / l
                            sc = pb_st.tile([128, NQS], f32, name="sc", tag="sc")
                            if NQS > 1:
                                negm = pb_st.tile([128, 1], f32, name="negm", tag="negm")
                                nc.vector.tensor_reduce(out=negm[:], in_=nm_pack[:],
                                                        axis=mybir.AxisListType.X,
                                                        op=mybir.AluOpType.min)
                                d_p = pb_st.tile([128, NQS], f32, name="d_p", tag="d_p")
                                nc.vector.tensor_scalar(out=d_p[:], in0=nm_pack[:],
                                                        scalar1=negm[:], scalar2=None,
                                                        op0=mybir.AluOpType.subtract)
                                e_p = pb_st.tile([128, NQS], f32, name="e_p", tag="e_p")
                                nc.scalar.activation(e_p[:], d_p[:],
                                                     mybir.ActivationFunctionType.Exp,
                                                     scale=-1.0)
                                lw = pb_st.tile([128, NQS], f32, name="lw", tag="lw")
                                nc.vector.tensor_mul(lw[:], l_pack[:], e_p[:])
                                l_tot = pb_st.tile([128, 1], f32, name="l_tot", tag="l_tot")
                                nc.vector.reduce_sum(out=l_tot[:], in_=lw[:],
                                                     axis=mybir.AxisListType.X)
                                rl = pb_st.tile([128, 1], f32, name="rl", tag="rl")
                                nc.vector.reciprocal(rl[:], l_tot[:])
                                nc.vector.tensor_scalar_mul(sc[:], e_p[:], rl[:])
                            else:
                                nc.vector.reciprocal(sc[:], l_pack[:])
                            p_n = pb_pn.tile([128, Nk], f16, name="p_n", tag="p_n")
                            for j4 in range(NQS):
                                nc.vector.tensor_scalar_mul(p_n[:, ts(j4, JQ)],
                                                            p_t[:, ts(j4, JQ)],
                                                            sc[:, j4:j4 + 1])
                            # blockwise transpose: pThb[p, k, tt, q] = p_n[q, k*128+p]
                            nc.sync.dma_start(pThb[:, :, tt, :], p_n[:], transpose=True)

                        if pending is not None:
                            emit_av(*pending)
                        pending = (g, r, h, qb, pThb)
                if pending is not None:
                    emit_av(*pending)

            # ---------------- Phase C: proj + residual + LayerNorm ----------------
            with tc.tile_pool(name="pc_w", bufs=1) as pc_w, \
                 tc.tile_pool(name="pc_sb", bufs=2) as pc_sb, \
                 tc.tile_pool(name="pc_st", bufs=3) as pc_st, \
                 tc.tile_pool(name="pc_ps", bufs=4, space="PSUM") as pc_ps, \
                 tc.tile_pool(name="pc_bc", bufs=3, space="PSUM") as pc_bc:

                # broadcast proj_b / gamma / beta across partitions via ones-matmul
                ones = pc_w.tile([1, 128], f32, name="ones", tag="ones")
                nc.gpsimd.memset(ones[:], 1.0)
                bc = []
                for vi in range(3):
                    vrow = pc_w.tile([1, C], f32, name=f"vrow{vi}", tag=f"vrow{vi}")
                    nc.sync.dma_start(vrow[:], vec_d.ap()[vi:vi + 1, :])
                    bct = pc_w.tile([128, C], f32, name=f"bc{vi}", tag=f"bc{vi}")
                    for j in range(0, C, 512):
                        w = min(512, C - j)
                        psb = pc_bc.tile([128, 512], f32, name="psb", tag="psb")
                        nc.tensor.matmul(psb[:, :w], ones[:], vrow[:, j:j + w],
                                         start=True, stop=True)
                        nc.scalar.copy(bct[:, j:j + w], psb[:, :w])
                    bc.append(bct)
                bias_bc, gam_bc, bet_bc = bc

                wpb = []
                for c in range(G):
                    wpc = pc_w.tile([128, C], f16, name=f"wpb{c}", tag=f"wpb{c}")
                    nc.sync.dma_start(wpc[:], wp_d.ap()[ts(c, 128), :])
                    wpb.append(wpc)

                eps_t = pc_w.tile([128, 1], f32, name="eps_t", tag="eps_t")
                nc.gpsimd.memset(eps_t[:], 1e-5)

                NSTAT = 256
                nsub = C // NSTAT
                for t in range(TQ):
                    pps = []
                    for j in range(0, C, 384):
                        w = min(384, C - j)
                        pp = pc_ps.tile([128, 384], f32, name="pp", tag="pp")
                        for c in range(G):
                            nc.tensor.matmul(pp[:, :w], x1t[c][:, ts(t, 128)], wpb[c][:, j:j + w],
                                             start=(c == 0), stop=(c == G - 1))
                        pps.append((j, w, pp))
                    xr = pc_sb.tile([128, C], f32, name="xr", tag="xr")
                    nc.sync.dma_start(xr[:], xk_d.ap()[ts(t, 128), :])
                    u = pc_sb.tile([128, C], f32, name="u", tag="u")
                    for (j, w, pp) in pps:
                        nc.vector.tensor_add(u[:, j:j + w], pp[:, :w], bias_bc[:, j:j + w])
                    nc.vector.tensor_add(u[:], u[:], xr[:])

                    stats = pc_st.tile([128, nsub, 6], f32, name="stats", tag="stats")
                    for s in range(nsub):
                        nc.vector.bn_stats(out=stats[:, s, :], in_=u[:, ts(s, NSTAT)])
                    mv = pc_st.tile([128, 2], f32, name="mv", tag="mv")
                    nc.vector.bn_aggr(out=mv[:], in_=stats[:])
                    rstd = pc_st.tile([128, 1], f32, name="rstd", tag="rstd")
                    nc.scalar.activation(rstd[:], mv[:, 1:2],
                                         mybir.ActivationFunctionType.Sqrt, bias=eps_t[:])
                    nc.vector.reciprocal(rstd[:], rstd[:])

                    of = pc_sb.tile([128, C], f32, name="of", tag="of")
                    nc.vector.tensor_scalar(out=of[:], in0=u[:], scalar1=mv[:, 0:1],
                                            scalar2=rstd[:], op0=mybir.AluOpType.subtract,
                                            op1=mybir.AluOpType.mult)
                    nc.vector.tensor_mul(of[:], of[:], gam_bc[:])
                    nc.vector.tensor_add(of[:], of[:], bet_bc[:])
                    nc.sync.dma_start(out_d.ap()[ts(t, 128), :], of[:])

    nc.compile()
    return nc


_CACHE = {}


def _get_program(cfg: Cfg):
    if cfg not in _CACHE:
        _CACHE[cfg] = build_program(cfg)
    return _CACHE[cfg]


def _split16(w):
    hi = w.astype(np.float16)
    lo = (w - hi.astype(np.float32)).astype(np.float16)
    return np.ascontiguousarray(hi), np.ascontiguousarray(lo)


def make_in_maps(x, qkv_w, proj_w, proj_b, ln_gamma, ln_beta, cfg: Cfg):
    """Host-side shard prep. Returns list of 8 in_maps."""
    C = cfg.C
    B = x.shape[0]
    wq_t = np.ascontiguousarray((qkv_w[0:C] * np.float32(cfg.D ** 0.5)).T)
    wk_t = np.ascontiguousarray(qkv_w[C:2 * C].T)
    wv_t = np.ascontiguousarray(qkv_w[2 * C:3 * C].T)
    wp_t = np.ascontiguousarray(proj_w.T)
    wq_hi, wq_lo = _split16(wq_t)
    wk_hi, wk_lo = _split16(wk_t)
    wv_h = wv_t.astype(np.float16)
    wp_h = wp_t.astype(np.float16)
    vecs = np.ascontiguousarray(np.stack([proj_b, ln_gamma, ln_beta]).astype(np.float32))
    in_maps = []
    for core in range(8):
        b, half = core // 2, core % 2
        b = min(b, B - 1)
        xb = np.asarray(x[b], dtype=np.float32)
        if half == 0:
            xkc = np.ascontiguousarray(xb)
        else:
            xkc = np.ascontiguousarray(np.concatenate([xb[cfg.Nq:], xb[:cfg.Nq]], axis=0))
        in_maps.append({"xk": xkc, "wq_hi": wq_hi, "wq_lo": wq_lo,
                        "wk_hi": wk_hi, "wk_lo": wk_lo, "wv_h": wv_h,
                        "wp_h": wp_h, "vecs": vecs})
    return in_maps


def kernel(x, qkv_w, proj_w, proj_b, ln_gamma, ln_beta):
    from concourse.bass_utils import run_bass_kernel_spmd

    cfg = Cfg()
    nc = _get_program(cfg)
    x = np.asarray(x, dtype=np.float32)
    in_maps = make_in_maps(x, np.asarray(qkv_w, np.float32), np.asarray(proj_w, np.float32),
                           np.asarray(proj_b, np.float32), np.asarray(ln_gamma, np.float32),
                           np.asarray(ln_beta, np.float32), cfg)
    res = run_bass_kernel_spmd(nc, in_maps, core_ids=list(range(8)))
    B, N, C = x.shape
    out = np.empty((B, N, C), dtype=np.float32)
    for core in range(8):
        b, half = core // 2, core % 2
        out[b, half * cfg.Nq:(half + 1) * cfg.Nq] = res.results[core]["out"]
    return out
